# revision 1
# baseline (speedup 1.0000x reference)
"""Trainium2 Bass kernel for nn_AttentionConv (B=4,H=W=64,C=128,heads=2).

Sharding: 8 cores = (batch b in 0..3) x (query-half qh in 0..1).
Each core computes full attention for its 2048 query pixels of batch b,
over all 4096 keys, both heads, plus the qkv and output 1x1-conv
projections.  No cross-core communication.

Key optimization vs the ACT-bound baseline (179.8us): the softmax exp
work (16.8M elements/core) is split between ScalarE ACT (exact exp) and
a custom 8-stage DVE op EXP2_PACK_ANT registered at import time.  The
DVE op computes an approximate G*2^(T/128) and packs it directly as
bf16 bits through an int16-convert write:
  u=T+M; I=u-M (rne to 128-multiples); F=T-I; y=I+c+aF^2+bF -> int16
with the quadratic fitted so each segment stays inside one bf16 binade
(max mult err 0.51%, rms 0.20% -- comparable to bf16 quantization
itself; verified bit-exact vs the numpy model on HW).  Scores arrive
pre-scaled by 128*log2(e)*C^-0.5 (folded into wq host-side); the ACT
path matches the DVE path's global factor G via bias=ln(G), and G
cancels exactly in the softmax normalization.  DVE takes even key
chunks (kc 2..30; 6..30 in qg0), ACT the rest; the last six chunks are
head-split across both engines to shorten the tail.

Pipeline structure (steady state ~810-860 ns per key-chunk, PE-paced):
 - st PSUM pool has 3 rotating [128,1024] slots (6 banks); o_ps h0/h1
   reuse 2 fixed banks across query groups; projection / outproj PSUM
   tiles borrow st slots.
 - PV lags the exp by 3 chunks (et bufs=7) so the PE never sits between
   an exp and the S^T that recycles its st slot.
 - normalize uses reciprocal_approx_fast over the full [128,QG] tile
   (custom-DVE ops require base_partition 0) and is spread at kc 0-1 of
   the next query group; outproj at kc 4, one fused [128,512] bias add
   at kc 6, out-DMA at kc 8.
 - ramp: x and the 4 concatenated weights load via few large f32 HWDGE
   DMAs on two queues (SP + Activation); casts to bf16 are spread
   across DVE/ACT/GpSimd; the v4 ones-fill is split DVE/GpSimd;
   projections
   for chunks 0-1 run pre-loop, 2-7 interleaved into qg0; per-chunk
   K/Q and V casts are single fused strided copies.

Per-core device algorithm (matmuls bf16, accumulate f32):
 - QT = wq'^T x^T -> [128(2h x 64d), 2048]  (wq' pre-scaled)
 - KT = wk^T x^T  -> [128, 4096]
 - V tiles v4all[128, 64, 128]: cols 0..63 = v_head, cols 64..127 = 1.0
   (the ones make the PV matmul also emit the softmax row-sum Z).
 - per query-group (512/head) x key-chunk (128): S^T = K Q^T in PSUM
   (the two heads' matmuls pair concurrently on disjoint row groups),
   weights = G*exp(S*scale) via ACT or DVE, O'^T += V''^T E^T.
 - normalize: resT = O'^T * recip_approx(Z); out = resT^T w_out + b_out.
"""

import numpy as np

import concourse.bass as bass
import concourse.tile as tile
from concourse.tile import add_dep_helper
from concourse import bacc, mybir
from concourse.bass_utils import run_bass_kernel_spmd

import concourse.dve_ops as dops
from concourse.dve_spec import Spec, lower, Src0, Src1, C0, C1, C2, Latch
from concourse.dve_uop import DveOpSpec

F32 = mybir.dt.float32
BF16 = mybir.dt.bfloat16
I16 = mybir.dt.int16

B = 4
C = 128
NPIX = 4096          # 64*64 pixels per batch
NQ = 2048            # queries per core (half batch)
HC = 64              # head dim
KC = 128             # key chunk
NKC = NPIX // KC     # 32
QG = 512             # query group (per head; ST tile packs both heads)
NQG = NQ // QG       # 4
N_CORES = 8

# --- exp approximation constants (see fit in session notes) ---
EXP2_A = 2.63111957e-3
EXP2_B = 1.00227837
EXP2_C = 16310.5756
EXP2_M = float(1.5 * 2**30)
EXP2_BA = float(np.float32(EXP2_B) / np.float32(EXP2_A))
EXP2_LNG = 0.35459771189588246        # ln(G): ACT-path bias
LN2_128 = float(np.log(2.0) / 128.0)  # ACT-path scale
# host-side score pre-scale folded into wq: T = 128*log2(e)*C^-0.5 * s
PRESCALE = float(128.0 * np.log2(np.e) * C ** -0.5)

OP_NAME = "EXP2_PACK_ANT"

# which key-chunks each query-group runs on DVE (rest go to ACT)
import os as _os
_DVE_ON = _os.environ.get("KERNEL_DVE", "1") == "1"
_SAFE = set(_os.environ.get("KERNEL_SAFE", "").split(","))
_QG0_DVE = tuple(range(6, 32, 2))
_QGN_DVE = tuple(range(2, 32, 2))
DVE_KCS = [
    _QG0_DVE if _DVE_ON else (),
    _QGN_DVE if _DVE_ON else (),
    _QGN_DVE if _DVE_ON else (),
    _QGN_DVE if _DVE_ON else (),
]

_CACHE = {}


def _exp2_ref(in0, in1, s0, s1, imm2):
    f32 = np.float32
    T = in0.astype(f32)
    u = f32(T + f32(s1))
    I = f32(u - f32(s1))
    F = f32(T - I)
    ba = np.asarray(in1, f32).reshape(in0.shape[0], -1)[:, :1]
    r = f32(F + ba)
    p = f32(F * r)
    Q = f32(p * f32(s0))
    return f32(f32(I + f32(imm2)) + Q)


def _register_exp2_op():
    for op in dops.OPS:
        if op.name == OP_NAME:
            return op
    u = Src0 + C1
    I = u - C1
    F = Src0 - I
    r = F + Latch(Src1)
    y = (I + C2) + (F * r) * C0
    spec = Spec(body=y, reference=_exp2_ref)
    row = dops._CUSTOM_DVE_ROW_BASE + len(dops.OPS)
    assert row < 0x20
    shas = {
        ver: DveOpSpec(
            name=OP_NAME, opcode=row, uops=lower(spec, ver=ver), rd1_en=True
        ).sha(ver)
        for ver in ("v3", "v4")
    }
    op = dops.DveOp(OP_NAME, spec, subdim=False, uops_sha=shas)
    dops.OPS.append(op)
    dops._SUB_OPCODE_FOR_NAME[OP_NAME] = row
    dops.CUSTOM_DVE_SPECS[OP_NAME] = spec
    return op


EXP2_OP = _register_exp2_op()


def _build_nc():
    nc = bacc.Bacc("TRN2", target_bir_lowering=False, debug=False)

    xt_d = nc.dram_tensor("xt", [C, NPIX], F32, kind="ExternalInput")
    w4_d = nc.dram_tensor("w4", [C, 512], F32, kind="ExternalInput")
    bo_d = nc.dram_tensor("bo", [1, C], F32, kind="ExternalInput")
    out_d = nc.dram_tensor("out", [NQ, C], F32, kind="ExternalOutput")

    Exp = mybir.ActivationFunctionType.Exp

    with tile.TileContext(nc) as tc:
        with (
            tc.tile_pool(name="const", bufs=1) as const,
            tc.tile_pool(name="stage", bufs=4) as stage,
            tc.tile_pool(name="et", bufs=7) as etp,
            tc.tile_pool(name="rz", bufs=2) as rzp,
            tc.tile_pool(name="osb", bufs=2) as osbp,
            tc.tile_pool(name="st", bufs=1, space="PSUM") as stp,
            tc.tile_pool(name="op", bufs=1, space="PSUM") as opp,
        ):
            # ---- persistent SBUF tensors
            xtball = const.tile([C, NPIX], BF16, tag="xtball")
            qt = [const.tile([128, 512], BF16, tag=f"qt{j}", name=f"qt{j}")
                  for j in range(4)]
            kt = [const.tile([128, 512], BF16, tag=f"kt{j}", name=f"kt{j}")
                  for j in range(8)]
            v4all = const.tile([128, NKC * 2, 128], BF16, tag="v4all")
            rt = const.tile([128, NQ], BF16, tag="rt")
            bias_bc = const.tile([128, 512], F32, tag="bias_bc")
            w4b = const.tile([C, 512], BF16, tag="w4b")
            wqb = w4b[:, 0:128]
            wkb = w4b[:, 128:256]
            wvb = w4b[:, 256:384]
            wob = w4b[:, 384:512]
            ones1 = const.tile([1, C], F32, tag="ones1")
            warm = const.tile([1, 2], F32, tag="warm")
            ba_t = const.tile([128, 1], F32, tag="ba_t")
            lng_t = const.tile([128, 1], F32, tag="lng_t")

            # dummy exp first: loads the ACT table set off the critical path
            nc.vector.memset(warm[:], 0.0)
            nc.scalar.activation(warm[:], warm[:], Exp)
            nc.vector.memset(ba_t[:], EXP2_BA)
            nc.vector.memset(lng_t[:], EXP2_LNG)

            # PE warm-up: dummy matmuls while DMAs run, so the HAM
            # clock-gate reaches K=8/8 before the real matmuls start
            junk = const.tile([C, 512], BF16, tag="junk")
            nc.vector.memset(junk[:], 0.25)
            wst = stp.tile([128, 2 * QG], F32, tag="st", bufs=3, name="warm_st")
            for w in range(6):
                nc.tensor.matmul(wst[:, 0:512], junk[:, 0:128], junk[:],
                                 start=True, stop=True)

            # inputs: DRAM f32 -> SBUF bf16 cast-DMAs via gpsimd SWDGE
            # f32 HWDGE DMAs (x chunk 0 and q/k weights first), casts
            # spread across DVE / ACT / GpSimd so no engine serializes the
            # ramp
            xsall = stage.tile([C, NPIX], F32, tag="xsall", name="xsall")
            xs = [xsall[:, j * 512:(j + 1) * 512] for j in range(8)]
            # two HWDGE queues: SP carries x0 + mid-x, the Activation
            # engine's queue carries the weights + upper-x in parallel
            nc.sync.dma_start(xs[0], xt_d[:, 0:512])
            w32 = stage.tile([C, 512], F32, tag="w32", name="w32")
            nc.scalar.dma_start(w32[:], w4_d[:])
            nc.sync.dma_start(xsall[:, 512:2048], xt_d[:, 512:2048])
            nc.scalar.dma_start(xsall[:, 2048:4096], xt_d[:, 2048:4096])
            nc.vector.tensor_copy(w4b[:], w32[:])
            cast_eng = [nc.vector, nc.vector, nc.gpsimd, nc.gpsimd,
                        nc.gpsimd, nc.gpsimd, nc.gpsimd, nc.gpsimd]
            for j in range(8):
                eng = cast_eng[j]
                dst = xtball[:, j * 512:(j + 1) * 512]
                if eng is nc.scalar:
                    nc.scalar.copy(dst, xs[j])
                else:
                    eng.tensor_copy(dst, xs[j])

            def emit_proj_kq(j):
                # QT/KT projections for one 512-pixel chunk; PSUM tiles
                # borrow ST-pool slots.
                p = stp.tile([128, 2 * QG], F32, tag="st", bufs=3,
                             name=f"pkq{j}")
                nc.tensor.matmul(p[:, 0:512], wkb,
                                 xtball[:, j * 512:(j + 1) * 512],
                                 start=True, stop=True)
                if j < 4:  # QT over local queries
                    nc.tensor.matmul(p[:, 512:1024], wqb,
                                     xtball[:, j * 512:(j + 1) * 512],
                                     start=True, stop=True)
                if j % 2 == 1 or "scopy" in _SAFE:
                    nc.vector.tensor_copy(kt[j][:], p[:, 0:512])
                else:
                    nc.scalar.copy(kt[j][:], p[:, 0:512])
                if j < 4:
                    nc.vector.tensor_copy(qt[j][:], p[:, 512:1024])

            def emit_proj_v(j):
                pv = stp.tile([128, 2 * QG], F32, tag="st", bufs=3,
                              name=f"pv{j}")
                for kq in range(4):   # V natural per key chunk of 128
                    k = j * 4 + kq
                    nc.tensor.matmul(
                        pv[:, kq * 128:(kq + 1) * 128],
                        xtball[:, j * 512 + kq * 128:
                               j * 512 + (kq + 1) * 128],
                        wvb, start=True, stop=True)
                dst = v4all[:, 8 * j:8 * j + 8, 0:64]
                src = pv[:, 0:512].rearrange("p (s d) -> p s d", d=64)
                if j % 2 == 1 or "scopy" in _SAFE:
                    nc.vector.tensor_copy(dst, src)
                else:
                    nc.scalar.copy(dst, src)

            def emit_proj_chunk(j):
                emit_proj_kq(j)
                emit_proj_v(j)

            # ramp ordering: K/Q of chunk 0 first (feeds first S^T),
            # then wv cast + V(0), chunk 1, then ones memset + wo cast
            emit_proj_kq(0)

            emit_proj_v(0)
            emit_proj_kq(1)
            # ones columns of v4all: first half on DVE (PV(0) needs it a
            # few iters in); second half on GpSimd after its x casts
            # (needed only from PV(16), ~20 iterations in)
            nc.vector.memset(v4all[:, 0:32, 64:128], 1.0)
            nc.gpsimd.memset(v4all[:, 32:64, 64:128], 1.0)
            emit_proj_v(1)
            bo32 = stage.tile([1, C], F32, tag="bo32")
            nc.sync.dma_start(bo32[:], bo_d[:])
            nc.vector.memset(ones1[:], 1.0)

            def emit_bias_bc():
                # fp32 ones-matmul broadcast of b_out; deferred into qg0 so
                # its fp32 LOW/HIGH passes don't sit ahead of the first S^T
                bps = stp.tile([128, 2 * QG], F32, tag="st", bufs=3,
                               name="bps")
                nc.tensor.matmul(bps[:, 0:C], ones1[:], bo32[:],
                                 start=True, stop=True)
                for i in range(4):
                    nc.vector.tensor_copy(bias_bc[:, i * 128:(i + 1) * 128],
                                          bps[:, 0:C])

            # ---- attention (software-pipelined across query groups) ----
            # st pool: 3 rotating [128,1024] PSUM slots (6 banks); o_ps
            # h0/h1 reuse tags o0/o1 every qg (2 banks).  PV lags the exp
            # by 3 key-chunks (et bufs=7) so the PE never sits between an
            # exp and the ST that recycles its st slot.
            def emit_norm(qg, o_ps, piece):
                q0 = qg * QG
                if piece == 0:
                    for h in range(2):
                        if "recip" in _SAFE:
                            rz = rzp.tile([64, QG], F32, tag=f"rz{h}",
                                          name=f"rz{h}_{qg}")
                            nc.vector.reciprocal(rz[:], o_ps[h][64:128, :])
                        else:
                            rz = rzp.tile([128, QG], F32, tag=f"rz{h}",
                                          name=f"rz{h}_{qg}")
                            nc.vector.reciprocal_approx_fast(rz[:],
                                                             o_ps[h][:])
                        o_ps.append(rz)
                else:
                    for h in range(2):
                        rz = o_ps[2 + h]
                        rzv = rz[:] if "recip" in _SAFE else rz[64:128, :]
                        nc.vector.tensor_mul(
                            rt[h * HC:(h + 1) * HC, q0:q0 + QG],
                            o_ps[h][0:64, :], rzv)

            def emit_outproj_mm(qg, anchor):
                q0 = qg * QG
                gp = stp.tile([128, 2 * QG], F32, tag="st", bufs=3,
                              name=f"gps_{qg}")
                for i in range(4):
                    mm = nc.tensor.matmul(
                        gp[:, i * 128:(i + 1) * 128],
                        rt[:, q0 + i * 128:q0 + (i + 1) * 128],
                        wob, start=True, stop=True)
                    if anchor is not None:
                        add_dep_helper(mm.ins, anchor.ins, False,
                                       "outproj after next-qg S^T")
                ob = osbp.tile([128, 512], F32, tag="osb", name=f"ob_{qg}")
                return gp, ob

            def emit_outproj_add(qg, gp, ob):
                nc.vector.tensor_add(ob[:], gp[:, 0:512], bias_bc[:])

            def emit_out_dma(qg, ob):
                q0 = qg * QG
                nc.sync.dma_start(
                    out_d[q0:q0 + QG, :].rearrange("(c r) w -> r c w", r=128),
                    ob[:].rearrange("p (c w) -> p c w", w=128))

            pending = None      # (qg, o_ps) awaiting normalize/outproj
            pending_out = None  # (qg, gp, ob)
            for qg in range(NQG):
                o_ps = [opp.tile([128, QG], F32, tag=f"o{h}",
                                 name=f"o_ps{h}_{qg}") for h in range(2)]
                ets = {}
                anchor_mm = None
                dve_set = DVE_KCS[qg]
                for kc in range(NKC + 3):
                    if kc < NKC:
                        st = stp.tile([128, 2 * QG], F32, tag="st",
                                      bufs=3, name=f"st_{qg}_{kc}")
                        ktt = kt[kc // 4]
                        ks = slice((kc % 4) * 128, (kc % 4 + 1) * 128)
                        for h in range(2):
                            hp = slice(h * HC, (h + 1) * HC)
                            mm = nc.tensor.matmul(
                                st[:, h * QG:(h + 1) * QG],
                                ktt[hp, ks], qt[qg][hp, :],
                                start=True, stop=True)
                            if kc == 3 and h == 0:
                                anchor_mm = mm
                        et = etp.tile([128, 2 * QG], BF16, tag="et",
                                      bufs=7, name=f"et_{qg}_{kc}")
                        split = (qg == NQG - 1 and kc >= 26)
                        if split:
                            nc.scalar.activation(et[:, 0:QG], st[:, 0:QG],
                                                 Exp, bias=lng_t[:],
                                                 scale=LN2_128)
                            nc.vector._custom_dve(
                                EXP2_OP,
                                out=et[:, QG:2 * QG].bitcast(I16),
                                in0=st[:, QG:2 * QG],
                                in1=ba_t[:], s0=EXP2_A, s1=EXP2_M,
                                imm2=EXP2_C)
                        elif kc in dve_set:
                            nc.vector._custom_dve(
                                EXP2_OP,
                                out=et[:].bitcast(I16),
                                in0=st[:],
                                in1=ba_t[:], s0=EXP2_A, s1=EXP2_M,
                                imm2=EXP2_C)
                        else:
                            nc.scalar.activation(et[:], st[:], Exp,
                                                 bias=lng_t[:],
                                                 scale=LN2_128)
                        ets[kc] = et
                    if kc >= 3:
                        pk = kc - 3
                        pet = ets.pop(pk)
                        for h in range(2):
                            nc.tensor.matmul(
                                o_ps[h][:], v4all[:, 2 * pk + h, :],
                                pet[:, h * QG:(h + 1) * QG],
                                start=(pk == 0), stop=(pk == NKC - 1))
                    if qg == 0 and kc in (0, 3, 6, 9, 12, 15):
                        emit_proj_chunk(2 + kc // 3)
                    if qg == 0 and kc == 18:
                        emit_bias_bc()
                    if pending is not None and kc in (0, 1):
                        emit_norm(pending[0], pending[1], kc)
                    if kc == 4 and pending is not None:
                        gp, ob = emit_outproj_mm(pending[0], anchor_mm)
                        pending_out = (pending[0], gp, ob)
                        pending = None
                    if pending_out is not None and kc == 6:
                        emit_outproj_add(pending_out[0], pending_out[1],
                                         pending_out[2])
                    if kc == 8 and pending_out is not None:
                        emit_out_dma(pending_out[0], pending_out[2])
                        pending_out = None
                pending = (qg, o_ps)
            # final epilogue
            for piece in range(2):
                emit_norm(pending[0], pending[1], piece)
            gp, ob = emit_outproj_mm(pending[0], None)
            emit_outproj_add(pending[0], gp, ob)
            emit_out_dma(pending[0], ob)

    nc.compile()
    return nc


def _prep_in_maps(x, w_qkv, w_out, b_out):
    x = np.asarray(x, dtype=np.float32).reshape(B, NPIX, C)
    w_qkv = np.asarray(w_qkv, dtype=np.float32)
    w_out = np.asarray(w_out, dtype=np.float32)
    b_out = np.asarray(b_out, dtype=np.float32)

    wq = np.concatenate([w_qkv[:, 0:64], w_qkv[:, 192:256]],
                        axis=1) * PRESCALE
    wk = np.concatenate([w_qkv[:, 64:128], w_qkv[:, 256:320]], axis=1)
    wv = np.concatenate([w_qkv[:, 128:192], w_qkv[:, 320:384]], axis=1)
    w4 = np.ascontiguousarray(
        np.concatenate([wq, wk, wv, w_out], axis=1, dtype=np.float32))
    bo = np.ascontiguousarray(b_out.reshape(1, C))

    in_maps = []
    for core in range(N_CORES):
        b, qh = core // 2, core % 2
        xbT = x[b].T                     # [C, NPIX]
        q0 = qh * NQ
        xt = np.ascontiguousarray(
            np.concatenate([xbT[:, q0:], xbT[:, :q0]], axis=1))
        in_maps.append({"xt": xt, "w4": w4, "bo": bo})
    return in_maps


def run(x, w_qkv, w_out, b_out, trace=False, **run_kwargs):
    if "nc" not in _CACHE:
        _CACHE["nc"] = _build_nc()
    nc = _CACHE["nc"]
    in_maps = _prep_in_maps(x, w_qkv, w_out, b_out)
    res = run_bass_kernel_spmd(nc, in_maps, core_ids=list(range(N_CORES)),
                               trace=trace, **run_kwargs)
    out = np.empty((B, NPIX, C), dtype=np.float32)
    for core in range(N_CORES):
        b, qh = core // 2, core % 2
        out[b, qh * NQ:(qh + 1) * NQ, :] = res.results[core]["out"]
    return out.reshape(B, 64, 64, C), res


def kernel(x, w_qkv, w_out, b_out):
    out, _ = run(x, w_qkv, w_out, b_out, trace=False)
    return out

